# revision 4
# baseline (speedup 1.0000x reference)
"""Multi-head attention (B=4, S=2048, D=1024, H=16) on 8 TRN2 NeuronCores.

Sharding: core c handles batch b = c//2 and query-half qh = c%2 (1024 query
rows); K/V projection for its batch replicated across the 2 cores sharing the
batch. Zero inter-core communication.

v3: keep the PE instruction stream gapless so DVFS stays at the 2.4GHz
p-state and exp (Act engine) overlaps fully:
  - Host pre-casts inputs and weights to bf16 (halves DMA, 1 cyc/row
    matmuls everywhere, bf16-identity PE transposes).
  - Phase order: K stage -> V stage (+ Q transposes) -> attention pipeline.
    Q-projection psum-groups and the qt0 O-projection are interleaved into
    the attention emission as PE filler, so score matmuls never leave the
    PE idle while exp catches up.
  - Transpose/projection evictions on DVE (bf16 PSUM reads get the 2x DVE
    mode); Act does only exp + early DMA issuance.
  - O-projection uses OT chunks as stationary (128-row weight loads hide
    under 512-row moving Wo), Wo preloaded once as bf16.
"""

import numpy as np
import ml_dtypes

import concourse.bacc as bacc
import concourse.mybir as mybir
import concourse.tile as tile
from concourse import bass_utils
from concourse.masks import make_identity

F32 = mybir.dt.float32
BF16 = mybir.dt.bfloat16
EXP = mybir.ActivationFunctionType.Exp
COPY = mybir.ActivationFunctionType.Copy
ADD = mybir.AluOpType.add

B, S, D, H = 4, 2048, 1024, 16
SQ = 1024          # query rows per core
P = 128
MC = D // P        # 8 m-chunks (contraction of projections)
DKC = D // P       # 8 dk-chunks
KC = S // P        # 16 key chunks
SCALE = 1.0 / 32.0  # 1/sqrt(D_K)
N_CORES = 8

_CACHED_NC = None


def build_nc():
    nc = bacc.Bacc("TRN2", target_bir_lowering=False, debug=False,
                   num_devices=N_CORES)
    q_in = nc.dram_tensor("q_in", [SQ, D], BF16, kind="ExternalInput")
    k_in = nc.dram_tensor("k_in", [S, D], BF16, kind="ExternalInput")
    v_in = nc.dram_tensor("v_in", [S, D], BF16, kind="ExternalInput")
    wq_d = nc.dram_tensor("wq", [D, D], BF16, kind="ExternalInput")
    wk_d = nc.dram_tensor("wk", [D, D], BF16, kind="ExternalInput")
    wv_d = nc.dram_tensor("wv", [D, D], BF16, kind="ExternalInput")
    wo_d = nc.dram_tensor("wo", [D, D], BF16, kind="ExternalInput")
    bq_d = nc.dram_tensor("bq", [D], F32, kind="ExternalInput")
    bk_d = nc.dram_tensor("bk", [D], F32, kind="ExternalInput")
    bv_d = nc.dram_tensor("bv", [D], F32, kind="ExternalInput")
    bo_d = nc.dram_tensor("bo", [D], F32, kind="ExternalInput")
    out_d = nc.dram_tensor("out", [SQ, D], F32, kind="ExternalOutput")

    with tile.TileContext(nc) as tc:
        with tc.tile_pool(name="const", bufs=1) as constp:
            ident_f = constp.tile([P, P], F32)
            make_identity(nc, ident_f[:])
            ident = constp.tile([P, P], BF16)
            nc.vector.tensor_copy(ident[:], ident_f[:])
            bq_t = constp.tile([P, MC], F32)
            nc.scalar.dma_start(bq_t[:], bq_d.ap().rearrange("(c p) -> p c", p=P))
            bk_t = constp.tile([P, MC], F32)
            nc.scalar.dma_start(bk_t[:], bk_d.ap().rearrange("(c p) -> p c", p=P))
            bv_t = constp.tile([P, MC], F32)
            nc.scalar.dma_start(bv_t[:], bv_d.ap().rearrange("(c p) -> p c", p=P))
            bo_f = constp.tile([1, D], F32)
            nc.scalar.dma_start(bo_f[:], bo_d.ap().unsqueeze(0))
            bo_bc = constp.tile([P, D], F32)
            nc.gpsimd.partition_broadcast(bo_bc[:], bo_f[:])

            _build_body(nc, tc, q_in, k_in, v_in, wq_d, wk_d, wv_d, wo_d,
                        bq_t, bk_t, bv_t, bo_bc, ident, out_d)
    nc.compile()
    return nc


def _load_w(nc, wpool, w_d, tag):
    """DMA bf16 weight row-chunks on the Act queue. 8 tiles [128, D]."""
    tiles = []
    for mm in range(MC):
        t = wpool.tile([P, D], BF16, tag=f"{tag}{mm}", name=f"wt_{tag}{mm}")
        nc.scalar.dma_start(t[:], w_d.ap()[mm * P:(mm + 1) * P, :])
        tiles.append(t)
    return tiles


def _stage_x_transposed(nc, x_d, n_rows, stg, ps_t, ident, xT):
    """DMA x_d row-groups, PE-transpose into xT tiles [P, n_rows] bf16.
    Evictions on DVE (2-byte PSUM read -> 2x mode)."""
    ngroups = n_rows // (4 * P)
    for g in range(ngroups):
        rows = []
        for j in range(4):
            r = g * 4 + j
            t = stg.tile([P, D], BF16, tag="xin", bufs=6)
            nc.sync.dma_start(t[:], x_d.ap()[r * P:(r + 1) * P, :])
            rows.append(t)
        for mm in range(MC):
            pst = ps_t.tile([P, 512], BF16, tag="pst")
            for j in range(4):
                nc.tensor.transpose(
                    pst[:, j * P:(j + 1) * P],
                    rows[j][:, mm * P:(mm + 1) * P], ident[:])
            nc.vector.tensor_copy(xT[mm][:, g * 512:(g + 1) * 512], pst[:])


def _project(nc, ps_p, w_t, xT, n_cols, out_tiles, b_t):
    """out_tiles[dk][:, :] = (w^T x^T)[dk-chunk] + b, evicted on DVE."""
    for nh in range(n_cols // 512):
        for dk in range(DKC):
            ps = ps_p.tile([P, 512], F32, tag="pp")
            for mm in range(MC):
                nc.tensor.matmul(
                    ps[:], w_t[mm][:, dk * P:(dk + 1) * P],
                    xT[mm][:, nh * 512:(nh + 1) * 512],
                    start=(mm == 0), stop=(mm == MC - 1))
            nc.vector.tensor_scalar_add(
                out_tiles[dk][:, nh * 512:(nh + 1) * 512], ps[:],
                b_t[:, dk:dk + 1])


def _normalize_pair(nc, OT, rp, bcp, bv_t, qs, pair, pv1, pv2):
    """Softmax-normalize both heads of a pair from fused PV psums
    (row 64 = sums) into OT bf16; odd head partition-shifted via DMA."""
    for hh, pvp in ((0, pv1), (1, pv2)):
        rb = rp.tile([P, 512], F32, tag="rb", bufs=2, name="rb")
        nc.vector.tensor_copy(rb[64:65, :], pvp[64:65, :])
        r0 = rp.tile([1, 512], F32, tag="r0", bufs=2, name="r0")
        nc.gpsimd.tensor_copy(r0[:], rb[64:65, :])
        rr = rp.tile([1, 512], F32, tag="rr", bufs=2, name="rr")
        nc.vector.reciprocal_approx_fast(rr[:], r0[:])
        bc = bcp.tile([64, 512], F32, tag="bc", name="bc")
        nc.gpsimd.partition_broadcast(bc[:], rr[:])
        if hh == 0:
            osl = OT[pair][0:64, qs]
            nc.vector.tensor_mul(osl, pvp[0:64, :], bc[:])
            nc.vector.tensor_scalar_add(osl, osl, bv_t[0:64, pair:pair + 1])
        else:
            tmp = bcp.tile([64, 512], BF16, tag="tmp", bufs=2, name="tmp")
            nc.vector.tensor_mul(tmp[:], pvp[0:64, :], bc[:])
            osl = OT[pair][64:128, qs]
            nc.sync.dma_start(osl, tmp[:])
            nc.vector.tensor_scalar_add(osl, osl, bv_t[64:128, pair:pair + 1])


def _build_body(nc, tc, q_in, k_in, v_in, wq_d, wk_d, wv_d, wo_d,
                bq_t, bk_t, bv_t, bo_bc, ident, out_d):
    with (
        tc.tile_pool(name="ktp", bufs=1) as ktp,
        tc.tile_pool(name="qtp", bufs=1) as qtp,
        tc.tile_pool(name="wop", bufs=1) as wop,
        tc.tile_pool(name="otp", bufs=1) as otp,
    ):
        KT = [ktp.tile([P, S], BF16, tag=f"kt{i}", name=f"kt{i}")
              for i in range(DKC)]
        QT = [qtp.tile([P, SQ], BF16, tag=f"qt{i}", name=f"qt{i}")
              for i in range(DKC)]
        OT = [otp.tile([P, SQ], BF16, tag=f"ot{i}", name=f"ot{i}")
              for i in range(DKC)]
        WO = _load_w(nc, wop, wo_d, "wo")

        # ---- stage K (transposes + full K projection) ----
        with (
            tc.tile_pool(name="xtk", bufs=1) as xtp,
            tc.tile_pool(name="wk", bufs=1) as wpool,
            tc.tile_pool(name="stgk", bufs=2) as stg,
            tc.tile_pool(name="psk_t", bufs=3, space="PSUM") as ps_t,
            tc.tile_pool(name="psk_p", bufs=3, space="PSUM") as ps_p,
        ):
            wk_t = _load_w(nc, wpool, wk_d, "wk")
            xkT = [xtp.tile([P, S], BF16, tag=f"xkt{i}", name=f"xkt{i}")
                   for i in range(MC)]
            _stage_x_transposed(nc, k_in, S, stg, ps_t, ident, xkT)
            _project(nc, ps_p, wk_t, xkT, S, KT, bk_t)

        # ---- persistent for pipeline: transposed q + wq ----
        with (
            tc.tile_pool(name="xtq", bufs=1) as xtqp,
            tc.tile_pool(name="wq", bufs=1) as wqp,
            tc.tile_pool(name="vp", bufs=1) as vp,
        ):
            xqT = [xtqp.tile([P, SQ], BF16, tag=f"xqt{i}", name=f"xqt{i}")
                   for i in range(MC)]
            wq_t = _load_w(nc, wqp, wq_d, "wq")
            DEXT = H * 65  # V_ext: 65 cols per head (64 V + ones)
            V = [vp.tile([P, DEXT], BF16, tag=f"v{i}", name=f"v{i}")
                 for i in range(KC)]

            # ---- stage V + Q transposes ----
            with (
                tc.tile_pool(name="vtt", bufs=1) as vtt,
                tc.tile_pool(name="wv", bufs=1) as wpool,
                tc.tile_pool(name="stgv", bufs=2) as stg,
                tc.tile_pool(name="psv_t", bufs=3, space="PSUM") as ps_t,
                tc.tile_pool(name="psv_p", bufs=3, space="PSUM") as ps_p,
            ):
                wv_t = _load_w(nc, wpool, wv_d, "wv")
                valT = [vtt.tile([P, 512], BF16, tag=f"vt{i}", name=f"vt{i}")
                        for i in range(MC)]
                ones16 = vtt.tile([P, H], BF16, name="ones16")
                nc.vector.memset(ones16[:], 1.0)

                ngroups = S // (4 * P)
                for g in range(ngroups):
                    rows = []
                    for j in range(4):
                        r = g * 4 + j
                        t = stg.tile([P, D], BF16, tag="xin", bufs=6)
                        nc.sync.dma_start(t[:], v_in.ap()[r * P:(r + 1) * P, :])
                        rows.append(t)
                    for mm in range(MC):
                        pst = ps_t.tile([P, 512], BF16, tag="pst")
                        for j in range(4):
                            nc.tensor.transpose(
                                pst[:, j * P:(j + 1) * P],
                                rows[j][:, mm * P:(mm + 1) * P], ident[:])
                        nc.vector.tensor_copy(valT[mm][:], pst[:])
                    for j in range(4):
                        sc = g * 4 + j
                        vx = V[sc].rearrange("p (h c) -> p h c", c=65)
                        nc.vector.tensor_copy(
                            vx[:, :, 64:65],
                            ones16[:].rearrange("p (h c) -> p h c", c=1))
                        for nh in range(2):
                            ps = ps_p.tile([P, 512], F32, tag="pp")
                            for mm in range(MC):
                                nc.tensor.matmul(
                                    ps[:], valT[mm][:, j * P:(j + 1) * P],
                                    wv_t[mm][:, nh * 512:(nh + 1) * 512],
                                    start=(mm == 0), stop=(mm == MC - 1))
                            nc.vector.tensor_copy(
                                vx[:, 8 * nh:8 * nh + 8, 0:64],
                                ps[:].rearrange("p (h c) -> p h c", c=64))

                # Q transposes at the tail of the V stage (psum pool reuse)
                _stage_x_transposed(nc, q_in, SQ, stg, ps_t, ident, xqT)

            # ---- attention pipeline with interleaved Q-proj / O-proj ----
            with (
                tc.tile_pool(name="ep", bufs=4) as ep,
                tc.tile_pool(name="bcp", bufs=2) as bcp,
                tc.tile_pool(name="rp", bufs=3) as rp,
                tc.tile_pool(name="fin", bufs=3) as finp,
                tc.tile_pool(name="ps_sc", bufs=2, space="PSUM") as ps_sc,
                tc.tile_pool(name="ps_pv", bufs=2, space="PSUM") as ps_pv,
                tc.tile_pool(name="ps_pj", bufs=2, space="PSUM") as ps_pj,
            ):
                # PE filler emitters: each call emits one psum-group of
                # independent matmul work (Q-projection chunk or O-proj
                # chunk), keeping the PE busy while exp catches up.
                def q_group(dk, nh):
                    def emit():
                        ps = ps_pj.tile([P, 512], F32, tag="pj")
                        for mm in range(MC):
                            nc.tensor.matmul(
                                ps[:], wq_t[mm][:, dk * P:(dk + 1) * P],
                                xqT[mm][:, nh * 512:(nh + 1) * 512],
                                start=(mm == 0), stop=(mm == MC - 1))
                        nc.vector.tensor_scalar_add(
                            QT[dk][:, nh * 512:(nh + 1) * 512], ps[:],
                            bq_t[:, dk:dk + 1])
                    return emit

                def o_group(sc, nh):
                    def emit():
                        ss = slice(sc * P, (sc + 1) * P)
                        ns = slice(nh * 512, (nh + 1) * 512)
                        fps = ps_pj.tile([P, 512], F32, tag="pj")
                        for dk in range(DKC):
                            nc.tensor.matmul(
                                fps[:], OT[dk][:, ss], WO[dk][:, ns],
                                start=(dk == 0), stop=(dk == DKC - 1))
                        ob = finp.tile([P, 512], F32, tag="ob")
                        nc.vector.tensor_tensor(ob[:], fps[:],
                                                bo_bc[:, ns], ADD)
                        nc.sync.dma_start(out_d.ap()[ss, ns], ob[:])
                    return emit

                # Q proj for dk=0 must precede pair 0's scores.
                q_group(0, 0)()
                q_group(0, 1)()
                # Remaining Q groups: dk 1..7 x nh 0..1, interleaved into
                # pairs 0..6 (dk=pair+1 finishes during pair's attention).
                filler = {}
                for pair in range(7):
                    dk = pair + 1
                    filler[(pair, 0)] = [q_group(dk, 0)]
                    filler[(pair, 1)] = [q_group(dk, 1)]
                # qt0's O-proj chunks run during pair 7 qt1.
                filler[(7, 1)] = [o_group(sc, nh)
                                  for sc in range(4) for nh in range(2)]

                for qt in range(SQ // 512):
                    qs = slice(qt * 512, (qt + 1) * 512)
                    for pair in range(H // 2):
                        fill = list(filler.get((pair, qt), []))
                        pv1 = ps_pv.tile([P, 512], F32, tag="pv")
                        pv2 = ps_pv.tile([P, 512], F32, tag="pv")
                        c1 = (2 * pair) * 65
                        c2 = (2 * pair + 1) * 65
                        for k2 in range(KC // 2):
                            ka = slice(2 * k2 * P, (2 * k2 + 1) * P)
                            kb = slice((2 * k2 + 1) * P, (2 * k2 + 2) * P)
                            s1 = ps_sc.tile([P, 1024], F32, tag="sc")
                            s2 = ps_sc.tile([P, 1024], F32, tag="sc")
                            nc.tensor.matmul(
                                s1[:, 0:512], KT[pair][0:64, ka],
                                QT[pair][0:64, qs],
                                start=True, stop=True, tile_position=(0, 0))
                            nc.tensor.matmul(
                                s2[:, 0:512], KT[pair][64:128, ka],
                                QT[pair][64:128, qs],
                                start=True, stop=True, tile_position=(64, 0))
                            nc.tensor.matmul(
                                s1[:, 512:1024], KT[pair][0:64, kb],
                                QT[pair][0:64, qs],
                                start=True, stop=True, tile_position=(0, 0))
                            nc.tensor.matmul(
                                s2[:, 512:1024], KT[pair][64:128, kb],
                                QT[pair][64:128, qs],
                                start=True, stop=True, tile_position=(64, 0))
                            e1 = ep.tile([P, 1024], BF16, tag="e")
                            e2 = ep.tile([P, 1024], BF16, tag="e")
                            nc.scalar.activation(e1[:], s1[:], EXP,
                                                 scale=SCALE)
                            nc.scalar.activation(e2[:], s2[:], EXP,
                                                 scale=SCALE)
                            first = k2 == 0
                            last = k2 == KC // 2 - 1
                            nc.tensor.matmul(
                                pv1[0:65, :], V[2 * k2][:, c1:c1 + 65],
                                e1[:, 0:512], start=first, stop=False)
                            nc.tensor.matmul(
                                pv2[0:65, :], V[2 * k2][:, c2:c2 + 65],
                                e2[:, 0:512], start=first, stop=False)
                            nc.tensor.matmul(
                                pv1[0:65, :], V[2 * k2 + 1][:, c1:c1 + 65],
                                e1[:, 512:1024], start=False, stop=last)
                            nc.tensor.matmul(
                                pv2[0:65, :], V[2 * k2 + 1][:, c2:c2 + 65],
                                e2[:, 512:1024], start=False, stop=last)
                            # PE filler between iters (before next scores
                            # stall on the exp pipeline).
                            if fill and k2 % 4 == 1:
                                fill.pop(0)()
                        while fill:
                            fill.pop(0)()
                        _normalize_pair(nc, OT, rp, bcp, bv_t,
                                        qs, pair, pv1, pv2)

                # qt1's O-projection tail.
                for sc in range(4, 8):
                    for nh in range(2):
                        o_group(sc, nh)()


def get_nc():
    global _CACHED_NC
    if _CACHED_NC is None:
        _CACHED_NC = build_nc()
    return _CACHED_NC


def _bf16(x):
    return np.ascontiguousarray(np.asarray(x, np.float32)).astype(
        ml_dtypes.bfloat16)


def run(inputs, **kwargs):
    """Run on 8 cores; returns (full_output, BassKernelResults)."""
    nc = get_nc()
    queries = _bf16(inputs["queries"])
    keys = _bf16(inputs["keys"])
    values = _bf16(inputs["values"])
    base = {
        "wq": _bf16(inputs["Wq"]),
        "wk": _bf16(inputs["Wk"]),
        "wv": _bf16(inputs["Wv"]),
        "wo": _bf16(inputs["Wo"]),
        "bq": np.ascontiguousarray(np.asarray(inputs["bq"], np.float32)),
        "bk": np.ascontiguousarray(np.asarray(inputs["bk"], np.float32)),
        "bv": np.ascontiguousarray(np.asarray(inputs["bv"], np.float32)),
        "bo": np.ascontiguousarray(np.asarray(inputs["bo"], np.float32)),
    }
    in_maps = []
    for c in range(N_CORES):
        b, qh = c // 2, c % 2
        m = dict(base)
        m["q_in"] = np.ascontiguousarray(queries[b, qh * SQ:(qh + 1) * SQ])
        m["k_in"] = keys[b]
        m["v_in"] = values[b]
        in_maps.append(m)
    res = bass_utils.run_bass_kernel_spmd(
        nc, in_maps, core_ids=list(range(N_CORES)), **kwargs)
    out = np.empty((B, S, D), np.float32)
    for c in range(N_CORES):
        b, qh = c // 2, c % 2
        out[b, qh * SQ:(qh + 1) * SQ] = res.results[c]["out"]
    return out, res


def kernel(**inputs):
    out, _ = run(inputs)
    return out


if __name__ == "__main__":
    rng = np.random.default_rng(0)
    ins = {
        "queries": rng.standard_normal((B, S, D), dtype=np.float32),
        "keys": rng.standard_normal((B, S, D), dtype=np.float32),
        "values": rng.standard_normal((B, S, D), dtype=np.float32),
        "Wq": (rng.standard_normal((D, D), dtype=np.float32) / 32),
        "bq": np.zeros(D, np.float32),
        "Wk": (rng.standard_normal((D, D), dtype=np.float32) / 32),
        "bk": np.zeros(D, np.float32),
        "Wv": (rng.standard_normal((D, D), dtype=np.float32) / 32),
        "bv": np.zeros(D, np.float32),
        "Wo": (rng.standard_normal((D, D), dtype=np.float32) / 32),
        "bo": np.zeros(D, np.float32),
    }
    out = kernel(**ins)
    print("out", out.shape, out.dtype, np.abs(out).mean())


# revision 9
# speedup vs baseline: 1.0144x; 1.0144x over previous
"""Multi-head attention (B=4, S=2048, D=1024, H=16) on 8 TRN2 NeuronCores.

Sharding: core c handles batch b = c//2 and query-half qh = c%2 (1024 query
rows); K/V projection for its batch replicated across the 2 cores sharing the
batch. Zero inter-core communication.

v3: keep the PE instruction stream gapless so DVFS stays at the 2.4GHz
p-state and exp (Act engine) overlaps fully:
  - Host pre-casts inputs and weights to bf16 (halves DMA, 1 cyc/row
    matmuls everywhere, bf16-identity PE transposes).
  - Phase order: K stage -> V stage (+ Q transposes) -> attention pipeline.
    Q-projection psum-groups and the qt0 O-projection are interleaved into
    the attention emission as PE filler, so score matmuls never leave the
    PE idle while exp catches up.
  - Transpose/projection evictions on DVE (bf16 PSUM reads get the 2x DVE
    mode); Act does only exp + early DMA issuance.
  - O-projection uses OT chunks as stationary (128-row weight loads hide
    under 512-row moving Wo), Wo preloaded once as bf16.
"""

import numpy as np
import ml_dtypes

import concourse.bacc as bacc
import concourse.mybir as mybir
import concourse.tile as tile
from concourse import bass_utils
from concourse.masks import make_identity

F32 = mybir.dt.float32
BF16 = mybir.dt.bfloat16
EXP = mybir.ActivationFunctionType.Exp
COPY = mybir.ActivationFunctionType.Copy
ADD = mybir.AluOpType.add

B, S, D, H = 4, 2048, 1024, 16
SQ = 1024          # query rows per core
P = 128
MC = D // P        # 8 m-chunks (contraction of projections)
DKC = D // P       # 8 dk-chunks
KC = S // P        # 16 key chunks
SCALE = 1.0 / 32.0  # 1/sqrt(D_K)
N_CORES = 8

_CACHED_NC = None


def build_nc():
    nc = bacc.Bacc("TRN2", target_bir_lowering=False, debug=False,
                   num_devices=N_CORES)
    q_in = nc.dram_tensor("q_in", [SQ, D], BF16, kind="ExternalInput")
    k_in = nc.dram_tensor("k_in", [S, D], BF16, kind="ExternalInput")
    v_in = nc.dram_tensor("v_in", [S, D], BF16, kind="ExternalInput")
    wq_d = nc.dram_tensor("wq", [D, D], BF16, kind="ExternalInput")
    wk_d = nc.dram_tensor("wk", [D, D], BF16, kind="ExternalInput")
    wv_d = nc.dram_tensor("wv", [D, D], BF16, kind="ExternalInput")
    wo_d = nc.dram_tensor("wo", [D, D], BF16, kind="ExternalInput")
    bq_d = nc.dram_tensor("bq", [D], F32, kind="ExternalInput")
    bk_d = nc.dram_tensor("bk", [D], F32, kind="ExternalInput")
    bv_d = nc.dram_tensor("bv", [D], F32, kind="ExternalInput")
    bo_d = nc.dram_tensor("bo", [D], F32, kind="ExternalInput")
    out_d = nc.dram_tensor("out", [SQ, D], F32, kind="ExternalOutput")

    with tile.TileContext(nc) as tc:
        with tc.tile_pool(name="const", bufs=1) as constp:
            ident_f = constp.tile([P, P], F32)
            make_identity(nc, ident_f[:])
            ident = constp.tile([P, P], BF16)
            nc.vector.tensor_copy(ident[:], ident_f[:])
            bq_t = constp.tile([P, MC], F32)
            nc.scalar.dma_start(bq_t[:], bq_d.ap().rearrange("(c p) -> p c", p=P))
            bk_t = constp.tile([P, MC], F32)
            nc.scalar.dma_start(bk_t[:], bk_d.ap().rearrange("(c p) -> p c", p=P))
            bv_t = constp.tile([P, MC], F32)
            nc.scalar.dma_start(bv_t[:], bv_d.ap().rearrange("(c p) -> p c", p=P))
            bo_f = constp.tile([1, D], F32)
            nc.scalar.dma_start(bo_f[:], bo_d.ap().unsqueeze(0))
            bo_bc = constp.tile([P, D], F32)
            nc.gpsimd.partition_broadcast(bo_bc[:], bo_f[:])

            _build_body(nc, tc, q_in, k_in, v_in, wq_d, wk_d, wv_d, wo_d,
                        bq_t, bk_t, bv_t, bo_bc, ident, out_d)
    nc.compile()
    return nc


def _load_w(nc, wpool, w_d, tag):
    """DMA bf16 weight row-chunks on the Act queue. 8 tiles [128, D]."""
    tiles = []
    for mm in range(MC):
        t = wpool.tile([P, D], BF16, tag=f"{tag}{mm}", name=f"wt_{tag}{mm}")
        nc.scalar.dma_start(t[:], w_d.ap()[mm * P:(mm + 1) * P, :])
        tiles.append(t)
    return tiles


def _stage_x_transposed(nc, x_d, n_rows, stg, ps_t, ident, xT):
    """DMA x_d row-groups, PE-transpose into xT tiles [P, n_rows] bf16.
    Evictions on DVE (2-byte PSUM read -> 2x mode)."""
    ngroups = n_rows // (4 * P)
    for g in range(ngroups):
        rows = []
        for j in range(4):
            r = g * 4 + j
            t = stg.tile([P, D], BF16, tag="xin", bufs=6)
            nc.sync.dma_start(t[:], x_d.ap()[r * P:(r + 1) * P, :])
            rows.append(t)
        for mm in range(MC):
            pst = ps_t.tile([P, 512], BF16, tag="pst")
            for j in range(4):
                nc.tensor.transpose(
                    pst[:, j * P:(j + 1) * P],
                    rows[j][:, mm * P:(mm + 1) * P], ident[:])
            nc.vector.tensor_copy(xT[mm][:, g * 512:(g + 1) * 512], pst[:])


def _project(nc, ps_p, w_t, xT, n_cols, out_tiles, b_t):
    """out_tiles[dk][:, :] = (w^T x^T)[dk-chunk] + b, evicted on DVE."""
    for nh in range(n_cols // 512):
        for dk in range(DKC):
            ps = ps_p.tile([P, 512], F32, tag="pp")
            for mm in range(MC):
                nc.tensor.matmul(
                    ps[:], w_t[mm][:, dk * P:(dk + 1) * P],
                    xT[mm][:, nh * 512:(nh + 1) * 512],
                    start=(mm == 0), stop=(mm == MC - 1))
            nc.vector.tensor_scalar_add(
                out_tiles[dk][:, nh * 512:(nh + 1) * 512], ps[:],
                b_t[:, dk:dk + 1])


def _normalize_pair(nc, OT, rp, bcp, bv_t, qs, pair, pv1, pv2):
    """Softmax-normalize both heads of a pair from fused PV psums
    (row 64 = sums) into OT bf16; odd head partition-shifted via DMA."""
    for hh, pvp in ((0, pv1), (1, pv2)):
        rb = rp.tile([P, 512], F32, tag="rb", bufs=2, name="rb")
        nc.vector.tensor_copy(rb[64:65, :], pvp[64:65, :])
        r0 = rp.tile([1, 512], F32, tag="r0", bufs=2, name="r0")
        nc.gpsimd.tensor_copy(r0[:], rb[64:65, :])
        rr = rp.tile([1, 512], F32, tag="rr", bufs=2, name="rr")
        nc.vector.reciprocal_approx_fast(rr[:], r0[:])
        bc = bcp.tile([64, 512], F32, tag="bc", name="bc")
        nc.gpsimd.partition_broadcast(bc[:], rr[:])
        if hh == 0:
            osl = OT[pair][0:64, qs]
            nc.vector.tensor_mul(osl, pvp[0:64, :], bc[:])
            nc.vector.tensor_scalar_add(osl, osl, bv_t[0:64, pair:pair + 1])
        else:
            tmp = bcp.tile([64, 512], BF16, tag="tmp", bufs=2, name="tmp")
            nc.vector.tensor_mul(tmp[:], pvp[0:64, :], bc[:])
            osl = OT[pair][64:128, qs]
            nc.sync.dma_start(osl, tmp[:])
            nc.vector.tensor_scalar_add(osl, osl, bv_t[64:128, pair:pair + 1])


def _build_body(nc, tc, q_in, k_in, v_in, wq_d, wk_d, wv_d, wo_d,
                bq_t, bk_t, bv_t, bo_bc, ident, out_d):
    with (
        tc.tile_pool(name="ktp", bufs=1) as ktp,
        tc.tile_pool(name="qtp", bufs=1) as qtp,
        tc.tile_pool(name="wop", bufs=1) as wop,
        tc.tile_pool(name="otp", bufs=1) as otp,
    ):
        KT = [ktp.tile([P, S], BF16, tag=f"kt{i}", name=f"kt{i}")
              for i in range(DKC)]
        QT = [qtp.tile([P, SQ], BF16, tag=f"qt{i}", name=f"qt{i}")
              for i in range(DKC)]
        OT = [otp.tile([P, SQ], BF16, tag=f"ot{i}", name=f"ot{i}")
              for i in range(DKC)]
        WO = [wop.tile([P, D], BF16, tag=f"wo{mm}", name=f"wt_wo{mm}")
              for mm in range(MC)]  # loaded later, after wv

        # ---- stage K (transposes + full K projection) ----
        with (
            tc.tile_pool(name="xtk", bufs=1) as xtp,
            tc.tile_pool(name="wk", bufs=1) as wpool,
            tc.tile_pool(name="stgk", bufs=2) as stg,
            tc.tile_pool(name="psk_t", bufs=3, space="PSUM") as ps_t,
            tc.tile_pool(name="psk_p", bufs=3, space="PSUM") as ps_p,
        ):
            wk_t = _load_w(nc, wpool, wk_d, "wk")
            xkT = [xtp.tile([P, S], BF16, tag=f"xkt{i}", name=f"xkt{i}")
                   for i in range(MC)]
            _stage_x_transposed(nc, k_in, S, stg, ps_t, ident, xkT)
            _project(nc, ps_p, wk_t, xkT, S, KT, bk_t)

        # ---- persistent for pipeline: transposed q + wq ----
        with (
            tc.tile_pool(name="xtq", bufs=1) as xtqp,
            tc.tile_pool(name="wq", bufs=1) as wqp,
            tc.tile_pool(name="vp", bufs=1) as vp,
        ):
            xqT = [xtqp.tile([P, SQ], BF16, tag=f"xqt{i}", name=f"xqt{i}")
                   for i in range(MC)]
            wq_t = _load_w(nc, wqp, wq_d, "wq")
            DEXT = H * 65  # V_ext: 65 cols per head (64 V + ones)
            V = [vp.tile([P, DEXT], BF16, tag=f"v{i}", name=f"v{i}")
                 for i in range(KC)]

            # ---- stage V + Q transposes ----
            with (
                tc.tile_pool(name="vtt", bufs=1) as vtt,
                tc.tile_pool(name="wv", bufs=1) as wpool,
                tc.tile_pool(name="stgv", bufs=2) as stg,
                tc.tile_pool(name="psv_t", bufs=3, space="PSUM") as ps_t,
                tc.tile_pool(name="psv_p", bufs=3, space="PSUM") as ps_p,
            ):
                wv_t = _load_w(nc, wpool, wv_d, "wv")
                for mm in range(MC):  # WO after wv on the Act DMA queue
                    nc.scalar.dma_start(WO[mm][:],
                                        wo_d.ap()[mm * P:(mm + 1) * P, :])
                valT = [vtt.tile([P, 512], BF16, tag=f"vt{i}", name=f"vt{i}")
                        for i in range(MC)]
                ones16 = vtt.tile([P, H], BF16, name="ones16")
                nc.vector.memset(ones16[:], 1.0)

                ngroups = S // (4 * P)
                for g in range(ngroups):
                    rows = []
                    for j in range(4):
                        r = g * 4 + j
                        t = stg.tile([P, D], BF16, tag="xin", bufs=6)
                        nc.sync.dma_start(t[:], v_in.ap()[r * P:(r + 1) * P, :])
                        rows.append(t)
                    for mm in range(MC):
                        pst = ps_t.tile([P, 512], BF16, tag="pst")
                        for j in range(4):
                            nc.tensor.transpose(
                                pst[:, j * P:(j + 1) * P],
                                rows[j][:, mm * P:(mm + 1) * P], ident[:])
                        nc.vector.tensor_copy(valT[mm][:], pst[:])
                    for j in range(4):
                        sc = g * 4 + j
                        vx = V[sc].rearrange("p (h c) -> p h c", c=65)
                        nc.vector.tensor_copy(
                            vx[:, :, 64:65],
                            ones16[:].rearrange("p (h c) -> p h c", c=1))
                        for nh in range(2):
                            ps = ps_p.tile([P, 512], F32, tag="pp")
                            for mm in range(MC):
                                nc.tensor.matmul(
                                    ps[:], valT[mm][:, j * P:(j + 1) * P],
                                    wv_t[mm][:, nh * 512:(nh + 1) * 512],
                                    start=(mm == 0), stop=(mm == MC - 1))
                            nc.vector.tensor_copy(
                                vx[:, 8 * nh:8 * nh + 8, 0:64],
                                ps[:].rearrange("p (h c) -> p h c", c=64))

                # Q transposes at the tail of the V stage (psum pool reuse)
                _stage_x_transposed(nc, q_in, SQ, stg, ps_t, ident, xqT)

            # ---- attention pipeline with interleaved Q-proj / O-proj ----
            with (
                tc.tile_pool(name="ep", bufs=4) as ep,
                tc.tile_pool(name="bcp", bufs=2) as bcp,
                tc.tile_pool(name="rp", bufs=3) as rp,
                tc.tile_pool(name="fin", bufs=3) as finp,
                tc.tile_pool(name="ps_sc", bufs=3, space="PSUM") as ps_sc,
                tc.tile_pool(name="ps_pv", bufs=2, space="PSUM") as ps_pv,
            ):
                # PE filler emitters: each call emits one psum-group of
                # independent matmul work (Q-projection chunk or O-proj
                # chunk), keeping the PE busy while exp catches up. They
                # borrow score-pool psum slots (no exp waits on them, so
                # the slot recycles via a fast DVE read).
                def q_group(dk, nh):
                    def emit():
                        ps = ps_sc.tile([P, 1024], F32, tag="sc")
                        for mm in range(MC):
                            nc.tensor.matmul(
                                ps[:, 0:512],
                                wq_t[mm][:, dk * P:(dk + 1) * P],
                                xqT[mm][:, nh * 512:(nh + 1) * 512],
                                start=(mm == 0), stop=(mm == MC - 1))
                        nc.vector.tensor_scalar_add(
                            QT[dk][:, nh * 512:(nh + 1) * 512], ps[:, 0:512],
                            bq_t[:, dk:dk + 1])
                    return emit

                def o_group(sc, nh):
                    def emit():
                        ss = slice(sc * P, (sc + 1) * P)
                        ns = slice(nh * 512, (nh + 1) * 512)
                        fps = ps_sc.tile([P, 1024], F32, tag="sc")
                        for dk in range(DKC):
                            nc.tensor.matmul(
                                fps[:, 0:512], OT[dk][:, ss], WO[dk][:, ns],
                                start=(dk == 0), stop=(dk == DKC - 1))
                        ob = finp.tile([P, 512], F32, tag="ob")
                        nc.vector.tensor_tensor(ob[:], fps[:, 0:512],
                                                bo_bc[:, ns], ADD)
                        nc.sync.dma_start(out_d.ap()[ss, ns], ob[:])
                    return emit

                # Q proj for dk=0 must precede pair 0's scores.
                q_group(0, 0)()
                q_group(0, 1)()
                # Remaining Q groups: dk 1..7 x nh 0..1, interleaved into
                # pairs 0..6 (dk=pair+1 finishes during pair's attention).
                filler = {}
                for pair in range(7):
                    dk = pair + 1
                    filler[(pair, 0)] = [q_group(dk, 0)]
                    filler[(pair, 1)] = [q_group(dk, 1)]
                # qt0's O-proj chunks run during pair 7 qt1.
                filler[(7, 1)] = [o_group(sc, nh)
                                  for sc in range(4) for nh in range(2)]

                for qt in range(SQ // 512):
                    qs = slice(qt * 512, (qt + 1) * 512)
                    for pair in range(H // 2):
                        fill = list(filler.get((pair, qt), []))
                        pv1 = ps_pv.tile([P, 512], F32, tag="pv")
                        pv2 = ps_pv.tile([P, 512], F32, tag="pv")
                        c1 = (2 * pair) * 65
                        c2 = (2 * pair + 1) * 65
                        for k2 in range(KC // 2):
                            ka = slice(2 * k2 * P, (2 * k2 + 1) * P)
                            kb = slice((2 * k2 + 1) * P, (2 * k2 + 2) * P)
                            s1 = ps_sc.tile([P, 1024], F32, tag="sc")
                            s2 = ps_sc.tile([P, 1024], F32, tag="sc")
                            nc.tensor.matmul(
                                s1[:, 0:512], KT[pair][0:64, ka],
                                QT[pair][0:64, qs],
                                start=True, stop=True, tile_position=(0, 0))
                            nc.tensor.matmul(
                                s1[:, 512:1024], KT[pair][0:64, kb],
                                QT[pair][0:64, qs],
                                start=True, stop=True, tile_position=(0, 0))
                            nc.tensor.matmul(
                                s2[:, 0:512], KT[pair][64:128, ka],
                                QT[pair][64:128, qs],
                                start=True, stop=True, tile_position=(64, 0))
                            nc.tensor.matmul(
                                s2[:, 512:1024], KT[pair][64:128, kb],
                                QT[pair][64:128, qs],
                                start=True, stop=True, tile_position=(64, 0))
                            e1 = ep.tile([P, 1024], BF16, tag="e")
                            e2 = ep.tile([P, 1024], BF16, tag="e")
                            nc.scalar.activation(e1[:], s1[:], EXP,
                                                 scale=SCALE)
                            nc.scalar.activation(e2[:], s2[:], EXP,
                                                 scale=SCALE)
                            first = k2 == 0
                            last = k2 == KC // 2 - 1
                            nc.tensor.matmul(
                                pv1[0:65, :], V[2 * k2][:, c1:c1 + 65],
                                e1[:, 0:512], start=first, stop=False)
                            nc.tensor.matmul(
                                pv2[0:65, :], V[2 * k2][:, c2:c2 + 65],
                                e2[:, 0:512], start=first, stop=False)
                            nc.tensor.matmul(
                                pv1[0:65, :], V[2 * k2 + 1][:, c1:c1 + 65],
                                e1[:, 512:1024], start=False, stop=last)
                            nc.tensor.matmul(
                                pv2[0:65, :], V[2 * k2 + 1][:, c2:c2 + 65],
                                e2[:, 512:1024], start=False, stop=last)
                            # PE filler between iters (before next scores
                            # stall on the exp pipeline).
                            if fill and k2 % 2 == 1:
                                fill.pop(0)()
                        while fill:
                            fill.pop(0)()
                        _normalize_pair(nc, OT, rp, bcp, bv_t,
                                        qs, pair, pv1, pv2)

                # qt1's O-projection tail.
                for sc in range(4, 8):
                    for nh in range(2):
                        o_group(sc, nh)()


def get_nc():
    global _CACHED_NC
    if _CACHED_NC is None:
        _CACHED_NC = build_nc()
    return _CACHED_NC


def _bf16(x):
    return np.ascontiguousarray(np.asarray(x, np.float32)).astype(
        ml_dtypes.bfloat16)


def run(inputs, **kwargs):
    """Run on 8 cores; returns (full_output, BassKernelResults)."""
    nc = get_nc()
    queries = _bf16(inputs["queries"])
    keys = _bf16(inputs["keys"])
    values = _bf16(inputs["values"])
    base = {
        "wq": _bf16(inputs["Wq"]),
        "wk": _bf16(inputs["Wk"]),
        "wv": _bf16(inputs["Wv"]),
        "wo": _bf16(inputs["Wo"]),
        "bq": np.ascontiguousarray(np.asarray(inputs["bq"], np.float32)),
        "bk": np.ascontiguousarray(np.asarray(inputs["bk"], np.float32)),
        "bv": np.ascontiguousarray(np.asarray(inputs["bv"], np.float32)),
        "bo": np.ascontiguousarray(np.asarray(inputs["bo"], np.float32)),
    }
    in_maps = []
    for c in range(N_CORES):
        b, qh = c // 2, c % 2
        m = dict(base)
        m["q_in"] = np.ascontiguousarray(queries[b, qh * SQ:(qh + 1) * SQ])
        m["k_in"] = keys[b]
        m["v_in"] = values[b]
        in_maps.append(m)
    res = bass_utils.run_bass_kernel_spmd(
        nc, in_maps, core_ids=list(range(N_CORES)), **kwargs)
    out = np.empty((B, S, D), np.float32)
    for c in range(N_CORES):
        b, qh = c // 2, c % 2
        out[b, qh * SQ:(qh + 1) * SQ] = res.results[c]["out"]
    return out, res


def kernel(**inputs):
    out, _ = run(inputs)
    return out


if __name__ == "__main__":
    rng = np.random.default_rng(0)
    ins = {
        "queries": rng.standard_normal((B, S, D), dtype=np.float32),
        "keys": rng.standard_normal((B, S, D), dtype=np.float32),
        "values": rng.standard_normal((B, S, D), dtype=np.float32),
        "Wq": (rng.standard_normal((D, D), dtype=np.float32) / 32),
        "bq": np.zeros(D, np.float32),
        "Wk": (rng.standard_normal((D, D), dtype=np.float32) / 32),
        "bk": np.zeros(D, np.float32),
        "Wv": (rng.standard_normal((D, D), dtype=np.float32) / 32),
        "bv": np.zeros(D, np.float32),
        "Wo": (rng.standard_normal((D, D), dtype=np.float32) / 32),
        "bo": np.zeros(D, np.float32),
    }
    out = kernel(**ins)
    print("out", out.shape, out.dtype, np.abs(out).mean())
